# revision 9
# baseline (speedup 1.0000x reference)
"""GPT-Neo self-attention on 8 NeuronCores (Trainium2, Bass/Tile).

Sharding: tensor-parallel over (batch, head-group). Core i handles batch
i//4 and head-group i%4 (3 of 12 heads). Each core computes a partial
out-projection [S, D]; the host sums the 4 partials per batch.

Per-core pipeline (B=2, S=2048, D=768, H=12, HD=64, 3 heads/core):
  qT,kT   = W @ x.T       heads 0,1 stacked on partitions 0:64/64:128
  v       = x @ WvT       natural [sk, 65] per head (col 64 == 1.0)
  sT      = k-stationary, q-moving -> scoresT [sk, sq]; all 3 heads of a
            key-block go to one 3-bank PSUM group
  pT      = exp(sT + pad_bias)   ONE fused ACT per key-block (3D AP over
            the 3 heads); diagonal blocks are column-shrunk and
            triangle-masked by a bf16 0/1 multiply AFTER exp
  oT      = [v|1].T @ pT  accumulated per head in PSUM; row 64 = denom
  onorm   = oT * recip(denom) (reciprocal_approx_fast + partition bcast)
  y      += onorm01.T @ Wo01  +  onorm2bo.T @ Wo2   (2 matmuls per tile)

Projections for chunk c+1/c+2 are emitted ahead so the tensor engine has
filler work while ScalarE grinds the exp stream.  Matmul operands bf16
(full PE rate); accumulation/softmax fp32.
"""

import numpy as np
import ml_dtypes
from contextlib import ExitStack

import concourse.bass as bass
from concourse import bacc
import concourse.mybir as mybir
import concourse.tile as tile
from concourse.bass_utils import run_bass_kernel_spmd

B, S, D, H = 2, 2048, 768, 12
HD = 64          # head dim
HPC = 3          # heads per core
NCORES = 8
NEG = -1.0e30
F32 = mybir.dt.float32
BF16 = mybir.dt.bfloat16

KT = D // 128    # 6 k-tiles over the model dim
SQT = S // 128   # 16 seq tiles of 128
CH = S // 512    # 4 seq chunks of 512


def build_nc():
    nc = bacc.Bacc(None, target_bir_lowering=False)

    xT = nc.declare_dram_parameter("xT", [D, S], BF16, isOutput=False)
    # cols 0:64 = q0, 64:128 = q1, 128:192 = k0, 192:256 = k1,
    #      256:320 = q2, 320:384 = k2
    wqk = nc.declare_dram_parameter("wqk", [D, HPC * 128], BF16, isOutput=False)
    wv = nc.declare_dram_parameter("wv", [D, HPC * HD], BF16, isOutput=False)
    # rows 0:64 = Wo cols of h0 (transposed), 64:128 = h1
    wo01 = nc.declare_dram_parameter("wo01", [128, D], BF16, isOutput=False)
    # rows 0:64 = h2, row 64 = bo (group-0 cores only)
    wo2 = nc.declare_dram_parameter("wo2", [65, D], BF16, isOutput=False)
    # causal triangle: maskt[k, u] = 1.0 if u >= k else 0.0
    maskt = nc.declare_dram_parameter("maskt", [128, 128], BF16, isOutput=False)
    # padding bias per key position, packed [128, 16] (col j = keys 128j..)
    pbias = nc.declare_dram_parameter("pbias", [128, SQT], F32, isOutput=False)
    y = nc.declare_dram_parameter("y", [S, D], F32, isOutput=True)

    with tile.TileContext(nc) as tc:
        with ExitStack() as ctx:
            persist = ctx.enter_context(tc.tile_pool(name="persist", bufs=1))
            work = ctx.enter_context(tc.tile_pool(name="work", bufs=2))
            psum = ctx.enter_context(
                tc.tile_pool(name="psum", bufs=1, space="PSUM"))

            # ---- persistent SBUF tiles ----
            xT_sb = [persist.tile([128, S], BF16, tag=f"xT{k}", name=f"xT{k}")
                     for k in range(KT)]
            wqk_sb = [persist.tile([128, HPC * 128], BF16, tag=f"wqk{k}",
                                   name=f"wqk{k}") for k in range(KT)]
            wv_sb = [persist.tile([128, HPC * HD], BF16, tag=f"wv{k}",
                                  name=f"wv{k}") for k in range(KT)]
            wo01_sb = persist.tile([128, D], BF16, tag="wo01", name="wo01")
            wo2_sb = persist.tile([65, D], BF16, tag="wo2", name="wo2")
            mask_sb = persist.tile([128, 128], BF16, tag="maskt", name="maskt")
            pb_sb = persist.tile([128, SQT], F32, tag="pb", name="pb")
            # heads 0,1 stacked on partitions (0:64 / 64:128); head 2 alone
            q01_sb = persist.tile([128, S], BF16, tag="q01", name="q01")
            k01_sb = persist.tile([128, S], BF16, tag="k01", name="k01")
            q2_sb = persist.tile([64, S], BF16, tag="q2", name="q2")
            k2_sb = persist.tile([64, S], BF16, tag="k2", name="k2")
            # v: [sk 128, j, h, 65]; col 64 of each (j,h) group == 1.0
            v4 = persist.tile([128, SQT, HPC, 65], BF16, tag="v4", name="v4")
            # exp(scores) buffered for the whole chunk: [sk, j, h, sq-col]
            pt4 = persist.tile([128, SQT, HPC, 512], BF16, tag="pt4",
                               name="pt4")
            onorm01 = persist.tile([128, S], BF16, tag="on01", name="on01")
            onorm2 = persist.tile([65, S], BF16, tag="on2", name="on2")
            dn_sb = persist.tile([65, S], F32, tag="dn", name="dn")
            rc_sb = persist.tile([65, S], F32, tag="rc", name="rc")

            # ---- input DMAs, all hoisted ----
            for c in range(CH):
                cs = slice(512 * c, 512 * (c + 1))
                for k in range(KT):
                    nc.sync.dma_start(out=xT_sb[k][:, cs],
                                      in_=xT[128 * k:128 * (k + 1), cs])
            for k in range(KT):
                nc.sync.dma_start(out=wqk_sb[k][:],
                                  in_=wqk[128 * k:128 * (k + 1), :])
                nc.sync.dma_start(out=wv_sb[k][:],
                                  in_=wv[128 * k:128 * (k + 1), :])
            nc.sync.dma_start(out=wo01_sb[:], in_=wo01[:, :])
            nc.sync.dma_start(out=wo2_sb[:], in_=wo2[:, :])
            nc.sync.dma_start(out=mask_sb[:], in_=maskt[:, :])
            nc.sync.dma_start(out=pb_sb[:], in_=pbias[:, :])
            nc.vector.memset(v4[:], 1.0)          # ones col survives v copies
            nc.vector.memset(onorm2[64:65, :], 1.0)   # bo contraction row
            nc.vector.memset(dn_sb[:], 1.0)       # recip-safe filler lanes

            def emit_qk_group(c, off):
                """One q/k projection group (6 accumulating MMs + copy)."""
                cs = slice(512 * c, 512 * (c + 1))
                ps = psum.tile([128, 512], F32, tag="op", bufs=2, name="psqk")
                for k in range(KT):
                    nc.tensor.matmul(
                        out=ps[:],
                        lhsT=wqk_sb[k][:, off:off + 128],
                        rhs=xT_sb[k][:, cs],
                        start=(k == 0), stop=(k == KT - 1))
                if off == 0:
                    nc.vector.tensor_copy(out=q01_sb[:, cs], in_=ps[:])
                elif off == 128:
                    nc.vector.tensor_copy(out=k01_sb[:, cs], in_=ps[:])
                else:
                    nc.vector.tensor_copy(out=q2_sb[:, cs], in_=ps[0:64, :])
                    nc.vector.tensor_copy(out=k2_sb[:, cs], in_=ps[64:128, :])

            def emit_v_group(j):
                """v projection for one sk-tile (6 accumulating MMs)."""
                pv = psum.tile([128, HPC, HD], F32, tag="op", bufs=2,
                               name="psv")
                for k in range(KT):
                    nc.tensor.matmul(
                        out=pv[:],
                        lhsT=xT_sb[k][:, 128 * j:128 * (j + 1)],
                        rhs=wv_sb[k][:],
                        start=(k == 0), stop=(k == KT - 1))
                nc.vector.tensor_copy(out=v4[:, j, :, 0:HD], in_=pv[:])

            def emit_outproj_tile(t):
                """out-projection for one 128-row sq tile + store."""
                ts_ = slice(128 * t, 128 * (t + 1))
                ot = work.tile([128, D], F32, tag="ot", name="ot")
                for n0, n1 in ((0, 512), (512, 768)):
                    pp = psum.tile([128, 512], F32, tag="op", bufs=2,
                                   name="pp")
                    nc.tensor.matmul(
                        out=pp[:, 0:n1 - n0], lhsT=onorm01[:, ts_],
                        rhs=wo01_sb[:, n0:n1], start=True, stop=False)
                    nc.tensor.matmul(
                        out=pp[:, 0:n1 - n0], lhsT=onorm2[:, ts_],
                        rhs=wo2_sb[:, n0:n1], start=False, stop=True)
                    nc.vector.tensor_copy(out=ot[:, n0:n1],
                                          in_=pp[:, 0:n1 - n0])
                nc.sync.dma_start(out=y[ts_, :], in_=ot[:])

            def emit_proj(c):
                for off in (0, 128, 256):
                    emit_qk_group(c, off)
                for j in range(4 * c, 4 * c + 4):
                    emit_v_group(j)

            emit_proj(0)
            emit_proj(1)

            for c in range(CH):
                cs = slice(512 * c, 512 * (c + 1))
                nj = 4 * c + 4      # causal: key blocks 0 .. nj-1

                # PE filler work woven between the exp-paced score/AV blocks:
                # the previous chunk's out-projection and the next chunk's
                # projections, spread evenly over this chunk's j-loop
                fillers = []
                if c >= 1:
                    for t in range(4 * (c - 1), 4 * (c - 1) + 4):
                        fillers.append(
                            (lambda t=t: emit_outproj_tile(t)))
                if 1 <= c <= 2:
                    for off in (0, 128, 256):
                        fillers.append(
                            (lambda c=c, off=off: emit_qk_group(c + 1, off)))
                    for j in range(4 * (c + 1), 4 * (c + 1) + 4):
                        fillers.append((lambda j=j: emit_v_group(j)))
                fpos = {}
                for i in range(len(fillers)):
                    fpos.setdefault((i + 1) * nj // (len(fillers) + 1),
                                    []).append(fillers[i])

                po = [psum.tile([65, 512], F32, tag=f"po{h}", bufs=1,
                                name=f"po{h}") for h in range(HPC)]

                def emit_av(j):
                    jr = j - 4 * c
                    lo = 128 * jr if jr >= 0 else 0
                    for h in range(HPC):
                        nc.tensor.matmul(
                            out=po[h][:, lo:512],
                            lhsT=v4[:, j, h, :],
                            rhs=pt4[:, j, h, lo:512],
                            start=(j == 0), stop=(j == nj - 1))

                # ---- attention: scores -> exp -> (mask); AV runs lag-1 ----
                for j in range(nj):
                    jr = j - 4 * c
                    lo = 128 * jr if jr >= 0 else 0   # shrunk col offset
                    qs = slice(512 * c + lo, 512 * (c + 1))

                    sg = psum.tile([128, HPC, 512], F32, tag="sg", bufs=1,
                                   name="sg")
                    nc.tensor.matmul(
                        out=sg[:, 0, lo:512],
                        lhsT=k01_sb[0:64, 128 * j:128 * (j + 1)],
                        rhs=q01_sb[0:64, qs], start=True, stop=True)
                    nc.tensor.matmul(
                        out=sg[:, 1, lo:512],
                        lhsT=k01_sb[64:128, 128 * j:128 * (j + 1)],
                        rhs=q01_sb[64:128, qs], start=True, stop=True)
                    nc.tensor.matmul(
                        out=sg[:, 2, lo:512],
                        lhsT=k2_sb[:, 128 * j:128 * (j + 1)],
                        rhs=q2_sb[:, qs], start=True, stop=True)

                    nc.scalar.activation(
                        out=pt4[:, j, :, lo:512], in_=sg[:, :, lo:512],
                        func=mybir.ActivationFunctionType.Exp,
                        bias=pb_sb[:, j:j + 1])
                    if jr >= 0:   # diagonal block: zero the upper triangle
                        for h in range(HPC):
                            nc.vector.tensor_mul(
                                out=pt4[:, j, h, lo:lo + 128],
                                in0=pt4[:, j, h, lo:lo + 128],
                                in1=mask_sb[:])
                    if j >= 1:
                        emit_av(j - 1)
                    for f in fpos.get(j, []):
                        f()
                emit_av(nj - 1)

                # ---- normalize: onorm = po * recip(denominator row) ----
                # denominator rows parked at partitions 0/32/64 (32-aligned)
                for h in range(HPC):
                    nc.vector.tensor_copy(out=dn_sb[32 * h:32 * h + 1, cs],
                                          in_=po[h][64:65, :])
                nc.vector.reciprocal_approx_fast(out=rc_sb[0:65, cs],
                                                 in_=dn_sb[0:65, cs])
                # stage the 3 recip rows side by side on partition 0, then
                # one gpsimd broadcast serves all 3 heads
                rst = work.tile([1, HPC * 512], F32, tag="rst", name="rst")
                for h in range(HPC):
                    nc.vector.tensor_copy(out=rst[:, 512 * h:512 * (h + 1)],
                                          in_=rc_sb[32 * h:32 * h + 1, cs])
                bc = work.tile([65, HPC * 512], F32, tag="bc", name="bc")
                nc.gpsimd.partition_broadcast(bc[:], rst[:])
                on_dst = (onorm01[0:64, cs], onorm01[64:128, cs],
                          onorm2[0:64, cs])
                for h in range(HPC):
                    nc.vector.tensor_mul(
                        out=on_dst[h], in0=po[h][0:64, :],
                        in1=bc[0:64, 512 * h:512 * (h + 1)])

            # last chunk's out-projection has no later chunk to weave into
            for t in range(12, 16):
                emit_outproj_tile(t)

    nc.compile()
    return nc


def make_inputs(x, attention_mask, Wq, Wk, Wv, Wo, bo):
    """Per-core input maps (host-side sharding)."""
    bf = ml_dtypes.bfloat16
    # causal triangle 0/1 mask (shared): pass iff query-col >= key-row
    kk = np.arange(128)[:, None]
    uu = np.arange(128)[None, :]
    maskt = (uu >= kk).astype(np.float32).astype(bf)

    in_maps = []
    for core in range(NCORES):
        b, g = core // 4, core % 4
        h0, h1, h2 = range(HPC * g, HPC * (g + 1))
        xTb = np.ascontiguousarray(x[b].T).astype(bf)
        wqk = np.empty((D, HPC * 128), np.float32)
        wqk[:, 0:64] = Wq[HD * h0:HD * (h0 + 1), :].T
        wqk[:, 64:128] = Wq[HD * h1:HD * (h1 + 1), :].T
        wqk[:, 128:192] = Wk[HD * h0:HD * (h0 + 1), :].T
        wqk[:, 192:256] = Wk[HD * h1:HD * (h1 + 1), :].T
        wqk[:, 256:320] = Wq[HD * h2:HD * (h2 + 1), :].T
        wqk[:, 320:384] = Wk[HD * h2:HD * (h2 + 1), :].T
        wv_ = Wv[HD * h0:HD * (h2 + 1), :].T.copy()        # [768, 192]
        wo01_ = Wo[:, HD * h0:HD * h0 + 128].T.copy()      # [128, 768]
        wo2_ = np.zeros((65, D), np.float32)
        wo2_[0:64, :] = Wo[:, HD * h2:HD * (h2 + 1)].T
        if g == 0:  # bo must enter the partial-sum exactly once per batch
            wo2_[64, :] = bo
        # padding bias per key position (additive, pre-exp)
        pb = ((1.0 - attention_mask[b].astype(np.float32)) * NEG)
        pbias = np.ascontiguousarray(pb.reshape(SQT, 128).T)
        in_maps.append({"xT": xTb, "wqk": wqk.astype(bf),
                        "wv": wv_.astype(bf), "wo01": wo01_.astype(bf),
                        "wo2": wo2_.astype(bf), "maskt": maskt,
                        "pbias": pbias})
    return in_maps


_NC_CACHE = {}


def _get_nc():
    if "nc" not in _NC_CACHE:
        _NC_CACHE["nc"] = build_nc()
    return _NC_CACHE["nc"]


def kernel(x, attention_mask, Wq, Wk, Wv, Wo, bo, _trace=False, _trace_kwargs=None):
    x = np.asarray(x, np.float32)
    attention_mask = np.asarray(attention_mask, np.float32)
    Wq, Wk, Wv, Wo, bo = (np.asarray(a, np.float32) for a in (Wq, Wk, Wv, Wo, bo))
    nc = _get_nc()
    in_maps = make_inputs(x, attention_mask, Wq, Wk, Wv, Wo, bo)
    res = run_bass_kernel_spmd(nc, in_maps, list(range(NCORES)),
                               trace=_trace, **(_trace_kwargs or {}))
    parts = [np.asarray(res.results[i]["y"]) for i in range(NCORES)]
    out = np.stack([sum(parts[0:4]), sum(parts[4:8])]).astype(np.float32)
    if _trace:
        return out, res
    return out


# revision 10
# speedup vs baseline: 1.0621x; 1.0621x over previous
"""GPT-Neo self-attention on 8 NeuronCores (Trainium2, Bass/Tile).

Sharding: tensor-parallel over (batch, head-group). Core i handles batch
i//4 and head-group i%4 (3 of 12 heads). Each core computes a partial
out-projection [S, D]; the host sums the 4 partials per batch.

Per-core pipeline (B=2, S=2048, D=768, H=12, HD=64, 3 heads/core):
  qT,kT   = W @ x.T       heads 0,1 stacked on partitions 0:64/64:128
  v       = x @ WvT       natural [sk, 65] per head (col 64 == 1.0)
  sT      = k-stationary, q-moving -> scoresT [sk, sq]; all 3 heads of a
            key-block go to one 3-bank PSUM group
  pT      = exp(sT + pad_bias)   ONE fused ACT per key-block (3D AP over
            the 3 heads); diagonal blocks are column-shrunk and
            triangle-masked by a bf16 0/1 multiply AFTER exp
  oT      = [v|1].T @ pT  accumulated per head in PSUM; row 64 = denom
  onorm   = oT * recip(denom) (reciprocal_approx_fast + partition bcast)
  y      += onorm01.T @ Wo01  +  onorm2bo.T @ Wo2   (2 matmuls per tile)

Projections for chunk c+1/c+2 are emitted ahead so the tensor engine has
filler work while ScalarE grinds the exp stream.  Matmul operands bf16
(full PE rate); accumulation/softmax fp32.
"""

import numpy as np
import ml_dtypes
from contextlib import ExitStack

import concourse.bass as bass
from concourse import bacc
import concourse.mybir as mybir
import concourse.tile as tile
from concourse.bass_utils import run_bass_kernel_spmd

B, S, D, H = 2, 2048, 768, 12
HD = 64          # head dim
HPC = 3          # heads per core
NCORES = 8
NEG = -1.0e30
F32 = mybir.dt.float32
BF16 = mybir.dt.bfloat16

KT = D // 128    # 6 k-tiles over the model dim
SQT = S // 128   # 16 seq tiles of 128
CH = S // 512    # 4 seq chunks of 512


def build_nc():
    nc = bacc.Bacc(None, target_bir_lowering=False)

    xT = nc.declare_dram_parameter("xT", [D, S], BF16, isOutput=False)
    # cols 0:64 = q0, 64:128 = q1, 128:192 = k0, 192:256 = k1,
    #      256:320 = q2, 320:384 = k2
    wqk = nc.declare_dram_parameter("wqk", [D, HPC * 128], BF16, isOutput=False)
    wv = nc.declare_dram_parameter("wv", [D, HPC * HD], BF16, isOutput=False)
    # rows 0:64 = Wo cols of h0 (transposed), 64:128 = h1
    wo01 = nc.declare_dram_parameter("wo01", [128, D], BF16, isOutput=False)
    # rows 0:64 = h2, row 64 = bo (group-0 cores only)
    wo2 = nc.declare_dram_parameter("wo2", [65, D], BF16, isOutput=False)
    # causal triangle: maskt[k, u] = 1.0 if u >= k else 0.0
    maskt = nc.declare_dram_parameter("maskt", [128, 128], BF16, isOutput=False)
    # padding bias per key position, packed [128, 16] (col j = keys 128j..)
    pbias = nc.declare_dram_parameter("pbias", [128, SQT], F32, isOutput=False)
    y = nc.declare_dram_parameter("y", [S, D], F32, isOutput=True)

    with tile.TileContext(nc) as tc:
        with ExitStack() as ctx:
            persist = ctx.enter_context(tc.tile_pool(name="persist", bufs=1))
            work = ctx.enter_context(tc.tile_pool(name="work", bufs=2))
            psum = ctx.enter_context(
                tc.tile_pool(name="psum", bufs=1, space="PSUM"))

            # ---- persistent SBUF tiles ----
            xT1 = persist.tile([128, KT, S], BF16, tag="xT1", name="xT1")
            wqk1 = persist.tile([128, KT, HPC * 128], BF16, tag="wqk1",
                                name="wqk1")
            wv1 = persist.tile([128, KT, HPC * HD], BF16, tag="wv1",
                               name="wv1")
            wo01_sb = persist.tile([128, D], BF16, tag="wo01", name="wo01")
            wo2_sb = persist.tile([65, D], BF16, tag="wo2", name="wo2")
            mask_sb = persist.tile([128, 128], BF16, tag="maskt", name="maskt")
            pb_sb = persist.tile([128, SQT], F32, tag="pb", name="pb")
            # heads 0,1 stacked on partitions (0:64 / 64:128); head 2 alone
            q01_sb = persist.tile([128, S], BF16, tag="q01", name="q01")
            k01_sb = persist.tile([128, S], BF16, tag="k01", name="k01")
            q2_sb = persist.tile([64, S], BF16, tag="q2", name="q2")
            k2_sb = persist.tile([64, S], BF16, tag="k2", name="k2")
            # v: [sk 128, j, h, 65]; col 64 of each (j,h) group == 1.0
            v4 = persist.tile([128, SQT, HPC, 65], BF16, tag="v4", name="v4")
            # exp(scores) buffered for the whole chunk: [sk, j, h, sq-col]
            pt4 = persist.tile([128, SQT, HPC, 512], BF16, tag="pt4",
                               name="pt4")
            onorm01 = persist.tile([128, S], BF16, tag="on01", name="on01")
            onorm2 = persist.tile([65, S], BF16, tag="on2", name="on2")
            dn_sb = persist.tile([65, S], F32, tag="dn", name="dn")
            rc_sb = persist.tile([65, S], F32, tag="rc", name="rc")

            # ---- input DMAs, all hoisted (few, large transfers) ----
            xT_r = xT[:, :].rearrange("(k p) s -> p k s", k=KT)
            wqk_r = wqk[:, :].rearrange("(k p) n -> p k n", k=KT)
            wv_r = wv[:, :].rearrange("(k p) n -> p k n", k=KT)
            for c in range(CH):
                cs = slice(512 * c, 512 * (c + 1))
                nc.sync.dma_start(out=xT1[:, :, cs], in_=xT_r[:, :, cs])
            nc.sync.dma_start(out=wqk1[:], in_=wqk_r[:])
            nc.sync.dma_start(out=wv1[:], in_=wv_r[:])
            nc.sync.dma_start(out=wo01_sb[:], in_=wo01[:, :])
            nc.sync.dma_start(out=wo2_sb[:], in_=wo2[:, :])
            nc.sync.dma_start(out=mask_sb[:], in_=maskt[:, :])
            nc.sync.dma_start(out=pb_sb[:], in_=pbias[:, :])
            nc.vector.memset(v4[:], 1.0)          # ones col survives v copies
            nc.vector.memset(onorm2[64:65, :], 1.0)   # bo contraction row
            nc.vector.memset(dn_sb[:], 1.0)       # recip-safe filler lanes

            def emit_qk_group(c, off):
                """One q/k projection group (6 accumulating MMs + copy)."""
                cs = slice(512 * c, 512 * (c + 1))
                ps = psum.tile([128, 512], F32, tag="s01", bufs=2, name="psqk")
                for k in range(KT):
                    nc.tensor.matmul(
                        out=ps[:],
                        lhsT=wqk1[:, k, off:off + 128],
                        rhs=xT1[:, k, cs],
                        start=(k == 0), stop=(k == KT - 1))
                if off == 0:
                    nc.vector.tensor_copy(out=q01_sb[:, cs], in_=ps[:])
                elif off == 128:
                    nc.vector.tensor_copy(out=k01_sb[:, cs], in_=ps[:])
                else:
                    nc.vector.tensor_copy(out=q2_sb[:, cs], in_=ps[0:64, :])
                    nc.vector.tensor_copy(out=k2_sb[:, cs], in_=ps[64:128, :])

            def emit_v_group(j):
                """v projection for one sk-tile (6 accumulating MMs)."""
                pv = psum.tile([128, HPC, HD], F32, tag="s01", bufs=2,
                               name="psv")
                for k in range(KT):
                    nc.tensor.matmul(
                        out=pv[:],
                        lhsT=xT1[:, k, 128 * j:128 * (j + 1)],
                        rhs=wv1[:, k, :],
                        start=(k == 0), stop=(k == KT - 1))
                nc.vector.tensor_copy(out=v4[:, j, :, 0:HD], in_=pv[:])

            def emit_outproj_tile(t):
                """out-projection for one 128-row sq tile + store."""
                ts_ = slice(128 * t, 128 * (t + 1))
                ot = work.tile([128, D], F32, tag="ot", name="ot")
                for n0, n1 in ((0, 512), (512, 768)):
                    pp = psum.tile([128, 512], F32, tag="s01", bufs=2,
                                   name="pp")
                    nc.tensor.matmul(
                        out=pp[:, 0:n1 - n0], lhsT=onorm01[:, ts_],
                        rhs=wo01_sb[:, n0:n1], start=True, stop=False)
                    nc.tensor.matmul(
                        out=pp[:, 0:n1 - n0], lhsT=onorm2[:, ts_],
                        rhs=wo2_sb[:, n0:n1], start=False, stop=True)
                    nc.vector.tensor_copy(out=ot[:, n0:n1],
                                          in_=pp[:, 0:n1 - n0])
                nc.sync.dma_start(out=y[ts_, :], in_=ot[:])

            def emit_proj(c):
                for off in (0, 128, 256):
                    emit_qk_group(c, off)
                for j in range(4 * c, 4 * c + 4):
                    emit_v_group(j)

            emit_proj(0)
            emit_proj(1)

            for c in range(CH):
                cs = slice(512 * c, 512 * (c + 1))
                nj = 4 * c + 4      # causal: key blocks 0 .. nj-1

                # PE filler work woven between the exp-paced score/AV blocks:
                # the previous chunk's out-projection and the next chunk's
                # projections, spread evenly over this chunk's j-loop
                fillers = []
                if c >= 1:
                    for t in range(4 * (c - 1), 4 * (c - 1) + 4):
                        fillers.append(
                            (lambda t=t: emit_outproj_tile(t)))
                if 1 <= c <= 2:
                    for off in (0, 128, 256):
                        fillers.append(
                            (lambda c=c, off=off: emit_qk_group(c + 1, off)))
                    for j in range(4 * (c + 1), 4 * (c + 1) + 4):
                        fillers.append((lambda j=j: emit_v_group(j)))
                fpos = {}
                for i in range(len(fillers)):
                    fpos.setdefault((i + 1) * nj // (len(fillers) + 1),
                                    []).append(fillers[i])

                po = [psum.tile([65, 512], F32, tag=f"po{h}", bufs=1,
                                name=f"po{h}") for h in range(HPC)]

                def emit_av(j):
                    jr = j - 4 * c
                    lo = 128 * jr if jr >= 0 else 0
                    for h in range(HPC):
                        nc.tensor.matmul(
                            out=po[h][:, lo:512],
                            lhsT=v4[:, j, h, :],
                            rhs=pt4[:, j, h, lo:512],
                            start=(j == 0), stop=(j == nj - 1))

                # ---- attention: scores -> exp -> (mask); AV runs lag-1 ----
                for j in range(nj):
                    jr = j - 4 * c
                    lo = 128 * jr if jr >= 0 else 0   # shrunk col offset
                    qs = slice(512 * c + lo, 512 * (c + 1))

                    s01 = psum.tile([128, 2, 512], F32, tag="s01", bufs=2,
                                    name="s01")
                    nc.tensor.matmul(
                        out=s01[:, 0, lo:512],
                        lhsT=k01_sb[0:64, 128 * j:128 * (j + 1)],
                        rhs=q01_sb[0:64, qs], start=True, stop=True)
                    nc.tensor.matmul(
                        out=s01[:, 1, lo:512],
                        lhsT=k01_sb[64:128, 128 * j:128 * (j + 1)],
                        rhs=q01_sb[64:128, qs], start=True, stop=True)
                    s2 = psum.tile([128, 512], F32, tag="s2", bufs=1,
                                   name="s2")
                    nc.tensor.matmul(
                        out=s2[:, lo:512],
                        lhsT=k2_sb[:, 128 * j:128 * (j + 1)],
                        rhs=q2_sb[:, qs], start=True, stop=True)

                    nc.scalar.activation(
                        out=pt4[:, j, 0:2, lo:512], in_=s01[:, :, lo:512],
                        func=mybir.ActivationFunctionType.Exp,
                        bias=pb_sb[:, j:j + 1])
                    nc.scalar.activation(
                        out=pt4[:, j, 2, lo:512], in_=s2[:, lo:512],
                        func=mybir.ActivationFunctionType.Exp,
                        bias=pb_sb[:, j:j + 1])
                    if jr >= 0:   # diagonal block: zero the upper triangle
                        for h in range(HPC):
                            nc.vector.tensor_mul(
                                out=pt4[:, j, h, lo:lo + 128],
                                in0=pt4[:, j, h, lo:lo + 128],
                                in1=mask_sb[:])
                    if j >= 1:
                        emit_av(j - 1)
                    for f in fpos.get(j, []):
                        f()
                emit_av(nj - 1)

                # ---- normalize: onorm = po * recip(denominator row) ----
                # denominator rows parked at partitions 0/32/64 (32-aligned)
                for h in range(HPC):
                    nc.vector.tensor_copy(out=dn_sb[32 * h:32 * h + 1, cs],
                                          in_=po[h][64:65, :])
                nc.vector.reciprocal_approx_fast(out=rc_sb[0:65, cs],
                                                 in_=dn_sb[0:65, cs])
                # stage the 3 recip rows side by side on partition 0, then
                # one gpsimd broadcast serves all 3 heads
                rst = work.tile([1, HPC * 512], F32, tag="rst", name="rst")
                for h in range(HPC):
                    nc.vector.tensor_copy(out=rst[:, 512 * h:512 * (h + 1)],
                                          in_=rc_sb[32 * h:32 * h + 1, cs])
                bc = work.tile([65, HPC * 512], F32, tag="bc", name="bc")
                nc.gpsimd.partition_broadcast(bc[:], rst[:])
                on_dst = (onorm01[0:64, cs], onorm01[64:128, cs],
                          onorm2[0:64, cs])
                for h in range(HPC):
                    nc.vector.tensor_mul(
                        out=on_dst[h], in0=po[h][0:64, :],
                        in1=bc[0:64, 512 * h:512 * (h + 1)])

            # last chunk's out-projection has no later chunk to weave into
            for t in range(12, 16):
                emit_outproj_tile(t)

    nc.compile()
    return nc


def make_inputs(x, attention_mask, Wq, Wk, Wv, Wo, bo):
    """Per-core input maps (host-side sharding)."""
    bf = ml_dtypes.bfloat16
    # causal triangle 0/1 mask (shared): pass iff query-col >= key-row
    kk = np.arange(128)[:, None]
    uu = np.arange(128)[None, :]
    maskt = (uu >= kk).astype(np.float32).astype(bf)

    in_maps = []
    for core in range(NCORES):
        b, g = core // 4, core % 4
        h0, h1, h2 = range(HPC * g, HPC * (g + 1))
        xTb = np.ascontiguousarray(x[b].T).astype(bf)
        wqk = np.empty((D, HPC * 128), np.float32)
        wqk[:, 0:64] = Wq[HD * h0:HD * (h0 + 1), :].T
        wqk[:, 64:128] = Wq[HD * h1:HD * (h1 + 1), :].T
        wqk[:, 128:192] = Wk[HD * h0:HD * (h0 + 1), :].T
        wqk[:, 192:256] = Wk[HD * h1:HD * (h1 + 1), :].T
        wqk[:, 256:320] = Wq[HD * h2:HD * (h2 + 1), :].T
        wqk[:, 320:384] = Wk[HD * h2:HD * (h2 + 1), :].T
        wv_ = Wv[HD * h0:HD * (h2 + 1), :].T.copy()        # [768, 192]
        wo01_ = Wo[:, HD * h0:HD * h0 + 128].T.copy()      # [128, 768]
        wo2_ = np.zeros((65, D), np.float32)
        wo2_[0:64, :] = Wo[:, HD * h2:HD * (h2 + 1)].T
        if g == 0:  # bo must enter the partial-sum exactly once per batch
            wo2_[64, :] = bo
        # padding bias per key position (additive, pre-exp)
        pb = ((1.0 - attention_mask[b].astype(np.float32)) * NEG)
        pbias = np.ascontiguousarray(pb.reshape(SQT, 128).T)
        in_maps.append({"xT": xTb, "wqk": wqk.astype(bf),
                        "wv": wv_.astype(bf), "wo01": wo01_.astype(bf),
                        "wo2": wo2_.astype(bf), "maskt": maskt,
                        "pbias": pbias})
    return in_maps


_NC_CACHE = {}


def _get_nc():
    if "nc" not in _NC_CACHE:
        _NC_CACHE["nc"] = build_nc()
    return _NC_CACHE["nc"]


def kernel(x, attention_mask, Wq, Wk, Wv, Wo, bo, _trace=False, _trace_kwargs=None):
    x = np.asarray(x, np.float32)
    attention_mask = np.asarray(attention_mask, np.float32)
    Wq, Wk, Wv, Wo, bo = (np.asarray(a, np.float32) for a in (Wq, Wk, Wv, Wo, bo))
    nc = _get_nc()
    in_maps = make_inputs(x, attention_mask, Wq, Wk, Wv, Wo, bo)
    res = run_bass_kernel_spmd(nc, in_maps, list(range(NCORES)),
                               trace=_trace, **(_trace_kwargs or {}))
    parts = [np.asarray(res.results[i]["y"]) for i in range(NCORES)]
    out = np.stack([sum(parts[0:4]), sum(parts[4:8])]).astype(np.float32)
    if _trace:
        return out, res
    return out
